# revision 37
# baseline (speedup 1.0000x reference)
"""MinLSTM Trainium2 kernel.

Full-input contract: kernel(**inputs) takes the complete (unsharded) numpy
inputs of the reference model and returns the full [B, T+1, H] float32 output.

Math (per batch b, channel h — identical to the reference's log-space scan,
computed in linear space; every quantity is positive so the linear recurrence
is numerically stable):
    a = x @ W_f + b_f ;  b = x @ W_i + b_i ;  c = x @ W_h + b_h
    f = sigmoid(softplus(-b) - softplus(-a))        # forget gate
    i = 1 - f                                       # input gate
    g = max(c + 0.5, sigmoid(c))                    # = exp(log_g(c))
    h_t = f_t * h_{t-1} + i_t * g_t,   h_{-1} = g(h_0)
    out[:, 0] = g(h_0); out[:, t+1] = h_t

Sharding: 8 cores, core c -> (sample b = c//2, H-half hh = c%2, 256 channels).
Fully independent cores, no collectives. Host pre-transposes x to xT so the
device contraction dim (D) lies on partitions; host assembles the output.

Device pipeline per 512-wide T-chunk: DMA xT tiles -> matmuls (W stationary,
xT moving, PSUM fp32, [h, t] layout) -> ScalarE softplus/sigmoid gates ->
VectorE elementwise -> tensor_tensor_scan (fp32 state) -> DMA out.
"""

from contextlib import ExitStack

import numpy as np
import ml_dtypes

import concourse.bacc as bacc
import concourse.tile as tile
import concourse.mybir as mybir
from concourse.bass_utils import run_bass_kernel_spmd

# ---- fused custom DVE op: r = ~1/(in0+in1) --------------------------------
# One 8-slice pass: x = in0+in1; nx = bitcast(~x) (exponent-flip seed);
# u = x*nx lands in [-4.5,-4]; r = nx * p2(u) with p2 a degree-2 minimax of
# 1/u on that interval. Max rel err ~5.2e-5. Replaces a GpSimd add +
# reciprocal_approx_fast pair.
import concourse.dve_ops as _dve_ops
from concourse.dve_spec import (Spec as _Spec, Src0 as _S0, Src1 as _S1,
                                C0 as _C0, C1 as _C1, C2 as _C2,
                                AluOp as _AluOp, Bin as _Bin, lower as _lower)
from concourse.dve_uop import DveOpSpec as _DveOpSpec
from concourse.dve_table_gen import dve_ver_for as _dve_ver_for

ADD_RECIP_CONSTS = {"s0": -0.01306049, "s1": -0.16652115, "imm2": -0.70710396}


def _register_add_recip():
    name = "ADD_RECIP_POLY2_ANT"
    if name in _dve_ops._SUB_OPCODE_FOR_NAME:
        return next(o for o in _dve_ops.OPS if o.name == name)
    _x = _S0 + _S1
    _nx = _Bin(_AluOp.BITWISE_NOT, _x, _x)
    _u = _x * _nx

    def _ref(in0, in1, c0, c1, c2):
        x = (np.asarray(in0, np.float32) + np.asarray(in1, np.float32))
        x = x.astype(np.float32)
        nx = (~x.view(np.int32)).view(np.float32)
        u = x * nx
        return ((u * c0 + c1) * u + c2) * nx

    spec = _Spec(body=((_u * _C0 + _C1) * _u + _C2) * _nx, reference=_ref)
    row = _dve_ops._CUSTOM_DVE_ROW_BASE + len(_dve_ops.OPS)
    assert row < 0x20
    ver = _dve_ver_for("TRN2")
    sha = _DveOpSpec(name=name, opcode=row, uops=_lower(spec, ver=ver),
                     rd1_en=True).sha(ver)
    op = _dve_ops.DveOp(name, spec, subdim=False, uops_sha={ver: sha})
    _dve_ops.OPS.append(op)
    _dve_ops.CUSTOM_DVE_SPECS[name] = spec
    _dve_ops._SUB_OPCODE_FOR_NAME[name] = row
    return op


_ADD_RECIP_OP = _register_add_recip()

BF = mybir.dt.bfloat16
F16 = mybir.dt.float16
F32 = mybir.dt.float32
F32R = mybir.dt.float32r
AF = mybir.ActivationFunctionType
OP = mybir.AluOpType

B, T, D, H = 4, 8192, 512, 512
NCORES = 8
HS = H // 2          # 256 channels per core
TC = 512             # T chunk width
NCH = T // TC        # 16 chunks
NDT = D // 128       # 4 contraction tiles
NHT = HS // 128      # 2 h-tiles per core

# Matmul input mode: "f32r" (fp32 data, full-rate replicated mode) or "bf16".
MM_MODE = "f32r"
# Gate tensor dtype on-chip (f16 halves DVE cost vs f32, ~8x less rounding
# than bf16; values are in [0, ~8] so fp16 range is ample).
GT = F16

_nc_cache = {}


def _build_nc(mm_mode=MM_MODE):
    mm_dt = F32R if mm_mode == "f32r" else BF
    nc = bacc.Bacc("TRN2", target_bir_lowering=False, debug=False,
                   num_devices=NCORES)
    xT = nc.dram_tensor("xT", [D, T], mm_dt, kind="ExternalInput")
    w = nc.dram_tensor("w", [D, 3 * HS], mm_dt, kind="ExternalInput")
    aux = nc.dram_tensor("aux", [128, NHT], F32, kind="ExternalInput")
    out = nc.dram_tensor("out", [HS, T], F32, kind="ExternalOutput")

    def mm_ap(t):
        return t

    with tile.TileContext(nc) as tc, ExitStack() as ctx:
        wpool = ctx.enter_context(tc.tile_pool(name="w", bufs=1))
        xpool = ctx.enter_context(tc.tile_pool(name="x", bufs=4))
        gpool = ctx.enter_context(tc.tile_pool(name="g", bufs=4))
        hpool = ctx.enter_context(tc.tile_pool(name="h", bufs=4))
        ppool = ctx.enter_context(tc.tile_pool(name="p", bufs=2, space="PSUM"))

        # weight/aux loads go out on the ACT HWDGE queue so the first x-chunk
        # loads (SP queue) run in parallel with them; one 3D-AP DMA covers all
        # four 128-row d-slices
        wts = []
        for dt_ in range(NDT):
            t_ = wpool.tile([128, 3 * HS], mm_dt, tag=f"w{dt_}", name=f"w{dt_}")
            nc.scalar.dma_start(t_[:], w[dt_ * 128:(dt_ + 1) * 128, :])
            wts.append(t_)
        auxt = wpool.tile([128, NHT], F32, tag="aux")
        nc.scalar.dma_start(auxt[:], aux[:])

        chunks = [(k * TC, TC) for k in range(NCH)]

        carry = [None] * NHT
        for ci, (t0, tw) in enumerate(chunks):
            tsl = slice(t0, t0 + tw)
            xts = []
            for dt_ in range(NDT):
                xt = xpool.tile([128, TC], mm_dt, tag=f"x{dt_}", name=f"x{dt_}")
                nc.sync.dma_start(xt[:, :tw], xT[dt_ * 128:(dt_ + 1) * 128, tsl])
                xts.append(xt[:, :tw])
            for ht in range(NHT):
                # f_pre and i_pre share one two-bank PSUM tile so a single
                # ScalarE sigmoid covers both
                pfi = ppool.tile([128, 2, TC], F32, tag="pre01", bufs=2)
                pc_t = ppool.tile([128, TC], F32, tag="pre2", bufs=3)
                for wi in range(3):
                    dst = pc_t[:, :tw] if wi == 2 else pfi[:, wi, :tw]
                    for dt_ in range(NDT):
                        c0 = wi * HS + ht * 128
                        nc.tensor.matmul(
                            dst, mm_ap(wts[dt_][:, c0:c0 + 128]),
                            mm_ap(xts[dt_]),
                            start=(dt_ == 0), stop=(dt_ == NDT - 1))
                pc = pc_t[:, :tw]  # h_pre

                # f = sa/(sa+sb), i = sb/(sa+sb)  (exactly the reference's
                # sigmoid(softplus-difference) gates); g = max(c+.5, sigmoid(c))
                sab = gpool.tile([128, 2, TC], F32, tag="sab", name="sab")
                nc.scalar.activation(sab[:, :, :tw], pfi[:, :, :tw], AF.Sigmoid)
                sa = sab[:, 0, :tw]
                sb = sab[:, 1, :tw]
                sg = gpool.tile([128, TC], GT, tag="sg", name="sg")[:, :tw]
                nc.scalar.activation(sg, pc, AF.Sigmoid)
                # g emitted first on DVE so the h_pre PSUM bank frees early
                g = gpool.tile([128, TC], GT, tag="g", name="g")[:, :tw]
                nc.vector.scalar_tensor_tensor(g, pc, 0.5, sg, OP.add, OP.max)
                r = gpool.tile([128, TC], F32, tag="r", name="r")[:, :tw]
                c = ADD_RECIP_CONSTS
                nc.vector._custom_dve(_ADD_RECIP_OP, out=r, in0=sa,
                                      in1=sb, s0=c["s0"], s1=c["s1"],
                                      imm2=c["imm2"])
                f = gpool.tile([128, TC], GT, tag="f", name="f")[:, :tw]
                nc.gpsimd.tensor_tensor(f, sa, r, op=OP.mult)
                # f + (1-f) = (sa+sb)/s: the input gate is exactly 1-f
                w = gpool.tile([128, TC], GT, tag="w", name="w")[:, :tw]
                nc.vector.tensor_scalar(w, f, -1.0, 1.0, OP.mult, OP.add)
                v = gpool.tile([128, TC], GT, tag="v", name="v")[:, :tw]
                nc.vector.tensor_tensor(v, w, g, op=OP.mult)
                h = hpool.tile([128, TC], F32, tag=f"h{ht}",
                               name=f"h{ht}")[:, :tw]
                ini = auxt[:, ht:ht + 1] if ci == 0 else carry[ht]
                nc.vector.tensor_tensor_scan(h, f, v, ini, OP.mult, OP.add)
                carry[ht] = h[:, tw - 1:tw]
                nc.sync.dma_start(out[ht * 128:(ht + 1) * 128, tsl], h)
    nc.compile()
    return nc


def _get_nc(mm_mode=MM_MODE):
    if mm_mode not in _nc_cache:
        _nc_cache[mm_mode] = _build_nc(mm_mode)
    return _nc_cache[mm_mode]


def _g_host(x):
    # exp(log_g(x)) of the reference, computed directly in fp32
    return np.where(x >= 0, x + 0.5, 1.0 / (1.0 + np.exp(-np.minimum(x, 0))))


def _run(inputs, mm_mode=MM_MODE, trace=False):
    x = np.asarray(inputs["x"], np.float32)
    h_0 = np.asarray(inputs["h_0"], np.float32)
    W_f = np.asarray(inputs["W_f"], np.float32)
    b_f = np.asarray(inputs["b_f"], np.float32)
    W_i = np.asarray(inputs["W_i"], np.float32)
    b_i = np.asarray(inputs["b_i"], np.float32)
    W_h = np.asarray(inputs["W_h"], np.float32)
    b_h = np.asarray(inputs["b_h"], np.float32)
    assert (b_f == 0).all() and (b_i == 0).all() and (b_h == 0).all(), \
        "device program folds zero biases"

    np_mm = np.float32 if mm_mode == "f32r" else ml_dtypes.bfloat16

    g0 = _g_host(h_0[:, 0, :])  # [B, H]
    xTs = [np.ascontiguousarray(x[b].T).astype(np_mm) for b in range(B)]

    in_maps = []
    for c in range(NCORES):
        b, hh = divmod(c, 2)
        hs = slice(hh * HS, (hh + 1) * HS)
        wcat = np.concatenate([W_f[:, hs], W_i[:, hs], W_h[:, hs]],
                              axis=1).astype(np_mm)
        auxa = np.ascontiguousarray(
            g0[b, hs].reshape(NHT, 128).T.astype(np.float32))
        in_maps.append({"xT": xTs[b], "w": wcat, "aux": auxa})

    nc = _get_nc(mm_mode)
    res = run_bass_kernel_spmd(nc, in_maps, core_ids=list(range(NCORES)),
                               trace=trace)

    out = np.empty((B, T + 1, H), np.float32)
    out[:, 0, :] = g0
    for c in range(NCORES):
        b, hh = divmod(c, 2)
        hs = slice(hh * HS, (hh + 1) * HS)
        out[b, 1:, hs] = res.results[c]["out"].T
    return out, res


def kernel(**inputs):
    out, _ = _run(inputs)
    return out


# revision 40
# speedup vs baseline: 1.0127x; 1.0127x over previous
"""MinLSTM Trainium2 kernel.

Full-input contract: kernel(**inputs) takes the complete (unsharded) numpy
inputs of the reference model and returns the full [B, T+1, H] float32 output.

Math (per batch b, channel h — identical to the reference's log-space scan,
computed in linear space; every quantity is positive so the linear recurrence
is numerically stable):
    a = x @ W_f + b_f ;  b = x @ W_i + b_i ;  c = x @ W_h + b_h
    f = sigmoid(softplus(-b) - softplus(-a))        # forget gate
    i = 1 - f                                       # input gate
    g = max(c + 0.5, sigmoid(c))                    # = exp(log_g(c))
    h_t = f_t * h_{t-1} + i_t * g_t,   h_{-1} = g(h_0)
    out[:, 0] = g(h_0); out[:, t+1] = h_t

Sharding: 8 cores, core c -> (sample b = c//2, H-half hh = c%2, 256 channels).
Fully independent cores, no collectives. Host pre-transposes x to xT so the
device contraction dim (D) lies on partitions; host assembles the output.

Device pipeline per 512-wide T-chunk: DMA xT tiles -> matmuls (W stationary,
xT moving, PSUM fp32, [h, t] layout) -> ScalarE softplus/sigmoid gates ->
VectorE elementwise -> tensor_tensor_scan (fp32 state) -> DMA out.
"""

from contextlib import ExitStack

import numpy as np
import ml_dtypes

import concourse.bacc as bacc
import concourse.tile as tile
import concourse.mybir as mybir
from concourse.bass_utils import run_bass_kernel_spmd

# ---- fused custom DVE op: r = ~1/(in0+in1) --------------------------------
# One 8-slice pass: x = in0+in1; nx = bitcast(~x) (exponent-flip seed);
# u = x*nx lands in [-4.5,-4]; r = nx * p2(u) with p2 a degree-2 minimax of
# 1/u on that interval. Max rel err ~5.2e-5. Replaces a GpSimd add +
# reciprocal_approx_fast pair.
import concourse.dve_ops as _dve_ops
from concourse.dve_spec import (Spec as _Spec, Src0 as _S0, Src1 as _S1,
                                C0 as _C0, C1 as _C1, C2 as _C2,
                                AluOp as _AluOp, Bin as _Bin, lower as _lower)
from concourse.dve_uop import DveOpSpec as _DveOpSpec
from concourse.dve_table_gen import dve_ver_for as _dve_ver_for

ADD_RECIP_CONSTS = {"s0": -0.01306049, "s1": -0.16652115, "imm2": -0.70710396}


def _register_add_recip():
    name = "ADD_RECIP_POLY2_ANT"
    if name in _dve_ops._SUB_OPCODE_FOR_NAME:
        return next(o for o in _dve_ops.OPS if o.name == name)
    _x = _S0 + _S1
    _nx = _Bin(_AluOp.BITWISE_NOT, _x, _x)
    _u = _x * _nx

    def _ref(in0, in1, c0, c1, c2):
        x = (np.asarray(in0, np.float32) + np.asarray(in1, np.float32))
        x = x.astype(np.float32)
        nx = (~x.view(np.int32)).view(np.float32)
        u = x * nx
        return ((u * c0 + c1) * u + c2) * nx

    spec = _Spec(body=((_u * _C0 + _C1) * _u + _C2) * _nx, reference=_ref)
    row = _dve_ops._CUSTOM_DVE_ROW_BASE + len(_dve_ops.OPS)
    assert row < 0x20
    ver = _dve_ver_for("TRN2")
    sha = _DveOpSpec(name=name, opcode=row, uops=_lower(spec, ver=ver),
                     rd1_en=True).sha(ver)
    op = _dve_ops.DveOp(name, spec, subdim=False, uops_sha={ver: sha})
    _dve_ops.OPS.append(op)
    _dve_ops.CUSTOM_DVE_SPECS[name] = spec
    _dve_ops._SUB_OPCODE_FOR_NAME[name] = row
    return op


_ADD_RECIP_OP = _register_add_recip()

BF = mybir.dt.bfloat16
F16 = mybir.dt.float16
F32 = mybir.dt.float32
F32R = mybir.dt.float32r
AF = mybir.ActivationFunctionType
OP = mybir.AluOpType

B, T, D, H = 4, 8192, 512, 512
NCORES = 8
HS = H // 2          # 256 channels per core
TC = 512             # T chunk width
NCH = T // TC        # 16 chunks
NDT = D // 128       # 4 contraction tiles
NHT = HS // 128      # 2 h-tiles per core

# Matmul input mode: "f32r" (fp32 data, full-rate replicated mode) or "bf16".
MM_MODE = "f32r"
# Gate tensor dtype on-chip (f16 halves DVE cost vs f32, ~8x less rounding
# than bf16; values are in [0, ~8] so fp16 range is ample).
GT = F16

_nc_cache = {}


def _build_nc(mm_mode=MM_MODE):
    mm_dt = F32R if mm_mode == "f32r" else BF
    nc = bacc.Bacc("TRN2", target_bir_lowering=False, debug=False,
                   num_devices=NCORES)
    xT = nc.dram_tensor("xT", [D, T], mm_dt, kind="ExternalInput")
    w = nc.dram_tensor("w", [D, 3 * HS], mm_dt, kind="ExternalInput")
    aux = nc.dram_tensor("aux", [128, NHT], F32, kind="ExternalInput")
    out = nc.dram_tensor("out", [HS, T], F32, kind="ExternalOutput")

    def mm_ap(t):
        return t

    with tile.TileContext(nc) as tc, ExitStack() as ctx:
        wpool = ctx.enter_context(tc.tile_pool(name="w", bufs=1))
        xpool = ctx.enter_context(tc.tile_pool(name="x", bufs=4))
        gpool = ctx.enter_context(tc.tile_pool(name="g", bufs=4))
        hpool = ctx.enter_context(tc.tile_pool(name="h", bufs=4))
        ppool = ctx.enter_context(tc.tile_pool(name="p", bufs=2, space="PSUM"))

        # weight/aux loads go out on the ACT HWDGE queue so the first x-chunk
        # loads (SP queue) run in parallel with them; one 3D-AP DMA covers all
        # four 128-row d-slices
        wts = []
        for dt_ in range(NDT):
            t_ = wpool.tile([128, 3 * HS], mm_dt, tag=f"w{dt_}", name=f"w{dt_}")
            nc.scalar.dma_start(t_[:], w[dt_ * 128:(dt_ + 1) * 128, :])
            wts.append(t_)
        auxt = wpool.tile([128, NHT], F32, tag="aux")
        nc.scalar.dma_start(auxt[:], aux[:])

        # chunk 0 split in half so the first matmul group starts on a
        # half-size x transfer
        chunks = [(0, TC // 2), (TC // 2, TC // 2)]
        chunks += [(k * TC, TC) for k in range(1, NCH)]

        carry = [None] * NHT
        for ci, (t0, tw) in enumerate(chunks):
            tsl = slice(t0, t0 + tw)
            xts = []
            for dt_ in range(NDT):
                xt = xpool.tile([128, TC], mm_dt, tag=f"x{dt_}", name=f"x{dt_}")
                nc.sync.dma_start(xt[:, :tw], xT[dt_ * 128:(dt_ + 1) * 128, tsl])
                xts.append(xt[:, :tw])
            for ht in range(NHT):
                # f_pre and i_pre share one two-bank PSUM tile so a single
                # ScalarE sigmoid covers both
                pfi = ppool.tile([128, 2, TC], F32, tag="pre01", bufs=2)
                pc_t = ppool.tile([128, TC], F32, tag="pre2", bufs=3)
                for wi in range(3):
                    dst = pc_t[:, :tw] if wi == 2 else pfi[:, wi, :tw]
                    for dt_ in range(NDT):
                        c0 = wi * HS + ht * 128
                        nc.tensor.matmul(
                            dst, mm_ap(wts[dt_][:, c0:c0 + 128]),
                            mm_ap(xts[dt_]),
                            start=(dt_ == 0), stop=(dt_ == NDT - 1))
                pc = pc_t[:, :tw]  # h_pre

                # f = sa/(sa+sb), i = sb/(sa+sb)  (exactly the reference's
                # sigmoid(softplus-difference) gates); g = max(c+.5, sigmoid(c))
                sab = gpool.tile([128, 2, TC], F32, tag="sab", name="sab")
                nc.scalar.activation(sab[:, :, :tw], pfi[:, :, :tw], AF.Sigmoid)
                sa = sab[:, 0, :tw]
                sb = sab[:, 1, :tw]
                sg = gpool.tile([128, TC], GT, tag="sg", name="sg")[:, :tw]
                nc.scalar.activation(sg, pc, AF.Sigmoid)
                # g emitted first on DVE so the h_pre PSUM bank frees early
                g = gpool.tile([128, TC], GT, tag="g", name="g")[:, :tw]
                nc.vector.scalar_tensor_tensor(g, pc, 0.5, sg, OP.add, OP.max)
                r = gpool.tile([128, TC], F32, tag="r", name="r")[:, :tw]
                c = ADD_RECIP_CONSTS
                nc.vector._custom_dve(_ADD_RECIP_OP, out=r, in0=sa,
                                      in1=sb, s0=c["s0"], s1=c["s1"],
                                      imm2=c["imm2"])
                f = gpool.tile([128, TC], GT, tag="f", name="f")[:, :tw]
                nc.gpsimd.tensor_tensor(f, sa, r, op=OP.mult)
                # f + (1-f) = (sa+sb)/s: the input gate is exactly 1-f
                w = gpool.tile([128, TC], GT, tag="w", name="w")[:, :tw]
                nc.vector.tensor_scalar(w, f, -1.0, 1.0, OP.mult, OP.add)
                v = gpool.tile([128, TC], GT, tag="v", name="v")[:, :tw]
                nc.vector.tensor_tensor(v, w, g, op=OP.mult)
                h = hpool.tile([128, TC], F32, tag=f"h{ht}",
                               name=f"h{ht}")[:, :tw]
                ini = auxt[:, ht:ht + 1] if ci == 0 else carry[ht]
                nc.vector.tensor_tensor_scan(h, f, v, ini, OP.mult, OP.add)
                carry[ht] = h[:, tw - 1:tw]
                nc.sync.dma_start(out[ht * 128:(ht + 1) * 128, tsl], h)
    nc.compile()
    return nc


def _get_nc(mm_mode=MM_MODE):
    if mm_mode not in _nc_cache:
        _nc_cache[mm_mode] = _build_nc(mm_mode)
    return _nc_cache[mm_mode]


def _g_host(x):
    # exp(log_g(x)) of the reference, computed directly in fp32
    return np.where(x >= 0, x + 0.5, 1.0 / (1.0 + np.exp(-np.minimum(x, 0))))


def _run(inputs, mm_mode=MM_MODE, trace=False):
    x = np.asarray(inputs["x"], np.float32)
    h_0 = np.asarray(inputs["h_0"], np.float32)
    W_f = np.asarray(inputs["W_f"], np.float32)
    b_f = np.asarray(inputs["b_f"], np.float32)
    W_i = np.asarray(inputs["W_i"], np.float32)
    b_i = np.asarray(inputs["b_i"], np.float32)
    W_h = np.asarray(inputs["W_h"], np.float32)
    b_h = np.asarray(inputs["b_h"], np.float32)
    assert (b_f == 0).all() and (b_i == 0).all() and (b_h == 0).all(), \
        "device program folds zero biases"

    np_mm = np.float32 if mm_mode == "f32r" else ml_dtypes.bfloat16

    g0 = _g_host(h_0[:, 0, :])  # [B, H]
    xTs = [np.ascontiguousarray(x[b].T).astype(np_mm) for b in range(B)]

    in_maps = []
    for c in range(NCORES):
        b, hh = divmod(c, 2)
        hs = slice(hh * HS, (hh + 1) * HS)
        wcat = np.concatenate([W_f[:, hs], W_i[:, hs], W_h[:, hs]],
                              axis=1).astype(np_mm)
        auxa = np.ascontiguousarray(
            g0[b, hs].reshape(NHT, 128).T.astype(np.float32))
        in_maps.append({"xT": xTs[b], "w": wcat, "aux": auxa})

    nc = _get_nc(mm_mode)
    res = run_bass_kernel_spmd(nc, in_maps, core_ids=list(range(NCORES)),
                               trace=trace)

    out = np.empty((B, T + 1, H), np.float32)
    out[:, 0, :] = g0
    for c in range(NCORES):
        b, hh = divmod(c, 2)
        hs = slice(hh * HS, (hh + 1) * HS)
        out[b, 1:, hs] = res.results[c]["out"].T
    return out, res


def kernel(**inputs):
    out, _ = _run(inputs)
    return out
